# revision 1
# baseline (speedup 1.0000x reference)
"""Trainium2 Bass kernel for CrossFrameAbsoluteAttn.

Math (per batch b, head h, with d=32, HW=4096):
    Q = Wq x2 + bq ; K = Wk x1 + bk ; V = Wv up(feat1) + bv
    sim = (Q^T K)/sqrt(d) ; t = Q^T wt + bt
    attn = relu((sim - t)/3000)             # no row normalization
    out = attn V^T ; out_feat = Wp out + bp
    ofd = avgpool2x2(out_feat) ; final = feat2 + ofd

Key folds used here:
  * sim - t = q'.k' with q' = [q; 1], k' = [k/sqrt(d) - wt; -bt]  (K=33 contraction)
  * 1/3000 and the 1/4 of the average pool are folded into V
  * avgpool2x2 commutes with the linear ops after the relu, so attn is pooled
    4:1 right after the relu and everything downstream runs at 32x32 res.
    Queries are ordered (dy, dx, y', x') so the pool is two contiguous
    half-tile adds: DVE fp16 TT (2x mode) then GPSIMD fp16 TT.
  * the tiny 1x1 convs / upsample / weight folds / final 4x288 projection are
    host-side numpy; the device does the quadratic part: scores matmul ->
    relu -> pool -> PV, emitting per-head pooled outputs [32, 256].

Sharding: 8 cores split the 8192 (b, query-row) space: core c owns batch c//4,
query rows 16*(c%4) .. 16*(c%4)+16 (1024 queries), all 9 heads. No collectives.
"""

import math
import sys

import numpy as np

sys.path.insert(0, "/opt/trn_rl_repo")

import concourse.bass as bass  # noqa: E402
import concourse.tile as tile  # noqa: E402
from concourse import bacc, mybir  # noqa: E402
from concourse import bass_utils  # noqa: E402

HEADS = 9
EMBED = 32
TEMPERATURE = 3000.0
B, H, W = 2, 64, 64
HW = H * W
N_CORES = 8
NQ = 1024           # queries per core
NG = NQ // 4        # pooled outputs per core
MT = HW // 128      # 32 m-tiles

F16 = mybir.dt.float16
F32 = mybir.dt.float32

_CACHE = {}


def _build(repeat=1):
    if ("nc", repeat) in _CACHE:
        return _CACHE[("nc", repeat)]

    nc = bacc.Bacc("TRN2", target_bir_lowering=False, debug=False,
                   num_devices=N_CORES)

    ka = nc.dram_tensor("ka", [33, HEADS * HW], F16, kind="ExternalInput").ap()
    qa = nc.dram_tensor("qa", [33, HEADS * NQ], F16, kind="ExternalInput").ap()
    vt = nc.dram_tensor("vt", [128, HEADS * MT * EMBED], F16,
                        kind="ExternalInput").ap()
    outh = nc.dram_tensor("outh", [HEADS * 32, NG], F16,
                          kind="ExternalOutput").ap()

    with tile.TileContext(nc) as tc:
        with (
            tc.tile_pool(name="const", bufs=1) as cpool,
            tc.tile_pool(name="attn", bufs=8) as apool,
            tc.tile_pool(name="poolx", bufs=8) as pxpool,
            tc.tile_pool(name="pooled", bufs=2 * MT + 2) as plpool,
            tc.tile_pool(name="outh", bufs=2) as opool,
            tc.tile_pool(name="ps_s", bufs=3, space="PSUM") as ps_s,
            tc.tile_pool(name="ps_pv", bufs=2, space="PSUM") as ps_pv,
        ):
            ka_s = cpool.tile([33, HEADS * HW], F16)
            qa_s = cpool.tile([33, HEADS * NQ], F16)
            vt_s = cpool.tile([128, HEADS * MT * EMBED], F16)
            # per-head, split across queues, in consumption order
            for h in range(HEADS):
                for part in range(4):
                    lo = h * HW + part * (HW // 4)
                    hi = lo + HW // 4
                    nc.sync.dma_start(ka_s[:, lo:hi], ka[:, lo:hi])
                nc.sync.dma_start(qa_s[:, h * NQ:(h + 1) * NQ],
                                  qa[:, h * NQ:(h + 1) * NQ])
                nc.sync.dma_start(
                    vt_s[:, h * MT * EMBED:(h + 1) * MT * EMBED],
                    vt[:, h * MT * EMBED:(h + 1) * MT * EMBED])

            pooled_tiles = {}

            def emit_scores(h):
                for mt in range(MT):
                    ps = ps_s.tile([128, NQ], F32)
                    lhsT = ka_s[:, h * HW + mt * 128: h * HW + (mt + 1) * 128]
                    nc.tensor.matmul(ps[:, 0:512], lhsT,
                                     qa_s[:, h * NQ: h * NQ + 512],
                                     start=True, stop=True)
                    nc.tensor.matmul(ps[:, 512:1024], lhsT,
                                     qa_s[:, h * NQ + 512: h * NQ + 1024],
                                     start=True, stop=True)
                    at = apool.tile([128, NQ], F16)
                    # relu engine split: 11/16 ACT, 5/16 DVE (DVE also pools),
                    # Bresenham-interleaved so neither engine starves
                    if ((h * MT + mt) * 11) % 16 < 11:
                        nc.scalar.activation(at[:], ps[:],
                                             mybir.ActivationFunctionType.Relu)
                    else:
                        nc.vector.tensor_scalar_max(at[:], ps[:], 0.0)
                    px = pxpool.tile([128, NQ // 2], F16, tag="px")
                    nc.vector.tensor_add(px[:], at[:, 0:NQ // 2],
                                         at[:, NQ // 2:NQ])
                    pl = plpool.tile([128, NG], F16, tag="pooled")
                    nc.gpsimd.tensor_add(pl[:], px[:, 0:NG], px[:, NG:2 * NG])
                    pooled_tiles[(h, mt)] = pl

            pv_psum = {}

            def emit_pv_mms(h):
                po = ps_pv.tile([32, NG], F32, tag="po")
                for mt in range(MT):
                    pl = pooled_tiles.pop((h, mt))
                    nc.tensor.matmul(
                        po[:],
                        vt_s[:, h * MT * EMBED + mt * EMBED:
                             h * MT * EMBED + (mt + 1) * EMBED],
                        pl[:], start=(mt == 0), stop=(mt == MT - 1),
                        skip_group_check=True)
                pv_psum[h] = po

            def emit_out(h):
                po = pv_psum.pop(h)
                oh = opool.tile([32, NG], F16, tag="oh")
                nc.vector.tensor_copy(oh[:], po[:])
                nc.sync.dma_start(outh[h * 32:(h + 1) * 32, :], oh[:])

            for _rep in range(repeat):
                emit_scores(0)
                for h in range(1, HEADS):
                    emit_pv_mms(h - 1)
                    emit_scores(h)
                    emit_out(h - 1)
                emit_pv_mms(HEADS - 1)
                emit_out(HEADS - 1)

    nc.compile()
    _CACHE[("nc", repeat)] = nc
    return nc


def _upsample2x(x):
    """[C,32,32] -> [C,64,64] bilinear, half-pixel centers, edge clamp."""
    C, h, w = x.shape
    idx = np.arange(2 * h)
    pos = 0.5 * idx - 0.25
    lo = np.floor(pos).astype(int)
    frac = (pos - lo).astype(np.float32)
    lo_c = np.clip(lo, 0, h - 1)
    hi_c = np.clip(lo + 1, 0, h - 1)
    y = x[:, lo_c, :] * (1 - frac)[None, :, None] + x[:, hi_c, :] * frac[None, :, None]
    z = y[:, :, lo_c] * (1 - frac)[None, None, :] + y[:, :, hi_c] * frac[None, None, :]
    return z.astype(np.float32)


def kernel(second_frame, first_frame_aligned, second_frame_feat,
           first_frame_feat_aligned, Wq, bq, Wk, bk, Wv, bv, Wp, bp, Wt, bt):
    second_frame = np.asarray(second_frame, np.float32)
    first_frame_aligned = np.asarray(first_frame_aligned, np.float32)
    second_frame_feat = np.asarray(second_frame_feat, np.float32)
    first_frame_feat_aligned = np.asarray(first_frame_feat_aligned, np.float32)
    Wq = np.asarray(Wq, np.float32); bq = np.asarray(bq, np.float32)
    Wk = np.asarray(Wk, np.float32); bk = np.asarray(bk, np.float32)
    Wv = np.asarray(Wv, np.float32); bv = np.asarray(bv, np.float32)
    Wp = np.asarray(Wp, np.float32); bp = np.asarray(bp, np.float32)
    Wt = np.asarray(Wt, np.float32); bt = np.asarray(bt, np.float32)

    nc = _build()

    s = 1.0 / math.sqrt(EMBED)
    wt = Wt[0]                                   # [32], shared across heads

    # per-batch host prep ----------------------------------------------------
    ka_b, qfull_b, vt_b = [], [], []
    for b in range(B):
        f1 = first_frame_aligned[b].reshape(3, HW)
        f2 = second_frame[b].reshape(3, HW)
        fu = _upsample2x(first_frame_feat_aligned[b]).reshape(4, HW)

        # K' per head: [33, HW]; rows k/sqrt(d)-wt, last row -bt
        kfull = (Wk * s) @ f1 + (bk * s)[:, None]          # [288, HW]
        kfull = kfull.reshape(HEADS, EMBED, HW) - wt[None, :, None]
        ka = np.empty((33, HEADS * HW), np.float16)
        for h in range(HEADS):
            ka[:EMBED, h * HW:(h + 1) * HW] = kfull[h]
            ka[EMBED, h * HW:(h + 1) * HW] = -bt[0]
        ka_b.append(ka)

        qfull_b.append((Wq @ f2 + bq[:, None]).reshape(HEADS, EMBED, H, W))

        # V'^T: [HW, 288] scaled; -> [p, h, mt, d]
        vtf = (fu.T @ Wv.T + bv[None, :]) / (TEMPERATURE * 4.0)   # [HW, 288]
        vtr = vtf.reshape(MT, 128, HEADS, EMBED)
        vt_b.append(np.ascontiguousarray(
            vtr.transpose(1, 2, 0, 3).reshape(128, HEADS * MT * EMBED)
        ).astype(np.float16))

    in_maps = []
    for c in range(N_CORES):
        b, y0 = c // 4, (c % 4) * 16
        # grouped query order: (dy, dx, y', x') -> pool = two half adds
        qc = qfull_b[b][:, :, y0:y0 + 16, :]               # [9, 32, 16, 64]
        qg = qc.reshape(HEADS, EMBED, 8, 2, 32, 2)         # y',dy,x',dx
        qg = qg.transpose(0, 1, 3, 5, 2, 4).reshape(HEADS, EMBED, NQ)
        qa = np.empty((33, HEADS * NQ), np.float16)
        for h in range(HEADS):
            qa[:EMBED, h * NQ:(h + 1) * NQ] = qg[h]
            qa[EMBED, h * NQ:(h + 1) * NQ] = 1.0
        in_maps.append({"ka": ka_b[b], "qa": qa, "vt": vt_b[b]})

    res = bass_utils.run_bass_kernel_spmd(nc, in_maps,
                                          core_ids=list(range(N_CORES)))

    out_ofd = np.zeros((B, 4, 32, 32), np.float32)
    for c in range(N_CORES):
        b, y0 = c // 4, (c % 4) * 16
        oh = res.results[c]["outh"].astype(np.float32)   # [288, 256]
        ofd = Wp @ oh + bp[:, None]                      # [4, 256]
        out_ofd[b, :, y0 // 2: y0 // 2 + 8, :] = ofd.reshape(4, 8, 32)
    out_fin = second_frame_feat + out_ofd
    return out_fin, out_ofd



# revision 10
# speedup vs baseline: 1.4645x; 1.4645x over previous
"""Trainium2 Bass kernel for CrossFrameAbsoluteAttn (v2: rank-4 scores + PV pool-fold).

Math (per batch b, head h, d=32, HW=4096):
    Q = Wq x2 + bq ; K = Wk x1 + bk ; V = Wv up(feat1) + bv
    scores = (Q^T K)/sqrt(d) - (Q^T wt + bt)     # [HW_q, HW_k]
    attn = relu(scores)/3000                     # no row normalization
    out = attn V^T ; out_feat = Wp out + bp
    ofd = avgpool2x2(out_feat) ; final = feat2 + ofd

Key structure exploited here:
  * Q and K are affine in the 3-channel frames, so scores factor through a
    4-dim contraction: scores_h[q,m] = q~_h[:,q] . k~[:,m] with
      k~[m]   = [x1_m; 1]                        (shared by ALL 9 heads)
      q~_h[q] = [A_h^T x2_q + u_h ; w_h.x2_q + c_h]
    where A_h = Wq_h^T Wk_h/s, u_h = Wk_h^T bq_h/s, w_h = Wq_h^T(bk_h/s - wt),
    c_h = bq_h.(bk_h/s) - wt.bq_h - bt, s = sqrt(32). K=4 matmuls (cost is
    set by streamed columns only, so the tiny contraction is free).
  * The 2x2 avgpool over queries commutes with everything after the relu, and
    is FOLDED INTO the PV matmul: with queries ordered (dy, dx, y', x'), the
    4 pool members sit at the same position of 4 contiguous 256-col blocks,
    which accumulate into one PSUM tile across the PV matmuls. Zero vector
    cost for pooling.
  * The kernel is PSUM-drain-bound: every score must leave PSUM through
    ScalarE (ACT) or VectorE (DVE), both ~1 elem/lane/cycle for f32 PSUM
    reads. Scores are produced into 3-bank [128,1536] PSUM tiles so the two
    relu drains run at large free-dim (low per-op overhead), split
    ACT [0:768] / DVE [768:1536].
  * PV uses PE column tiling: 4 heads' PV matmuls issued to distinct 32-col
    groups (tile_position=(0,32j)) run concurrently; PV is interleaved
    between score passes to hide in the PE's drain-wait gaps.
  * relu scale 2^-7 keeps attn in fp16 normal range; V carries 2^7/(3000*4).
  * host side: tiny per-head 4-dim affine prep, upsample, and the final 4x288
    projection (as in the reference harness contract).

Sharding: 8 cores split the (batch, query-row) space: core c owns batch c//4,
query rows 16*(c%4)..16*(c%4)+15 (1024 queries), all 9 heads. No collectives.
"""

import math
import sys

import numpy as np

sys.path.insert(0, "/opt/trn_rl_repo")

import concourse.bass as bass  # noqa: E402
import concourse.tile as tile  # noqa: E402
from concourse import bacc, mybir  # noqa: E402
from concourse import bass_utils  # noqa: E402

HEADS = 9
EMBED = 32
TEMPERATURE = 3000.0
B, H, W = 2, 64, 64
HW = H * W
N_CORES = 8
NQ = 1024            # queries per core
NG = NQ // 4         # pooled outputs per core
MT = HW // 128       # 32 key m-tiles
NTRI = 11            # m-tile triples (last has 2)
ASCALE = 2.0 ** -7   # attn fp16 scale (relu side)

F16 = mybir.dt.float16
F32 = mybir.dt.float32

HGROUPS = [(0, 1, 2, 3), (4, 5, 6, 7), (8,)]

_CACHE = {}


def _build(repeat=1, mode="full"):
    # mode: "full" | "drain" (no PV) | "scores" (matmuls only) — for timing
    # component isolation; only "full" computes the real output.
    if ("nc", repeat, mode) in _CACHE:
        return _CACHE[("nc", repeat, mode)]

    nc = bacc.Bacc("TRN2", target_bir_lowering=False, debug=False,
                   num_devices=N_CORES)

    # q~/k~ replicated on 3 partition groups (0-3, 32-35, 64-67) so the 3
    # score matmuls of a pass go to distinct PE row groups and run
    # concurrently (row tiling).
    qa = nc.dram_tensor("qa", [12, HEADS * NQ], F16, kind="ExternalInput").ap()
    ka = nc.dram_tensor("ka", [12, NTRI * 128], F16, kind="ExternalInput").ap()
    vt = nc.dram_tensor("vt", [128, HEADS * MT * EMBED], F16,
                        kind="ExternalInput").ap()
    outh = nc.dram_tensor("outh", [HEADS * 32, NG], F16,
                          kind="ExternalOutput").ap()

    Relu = mybir.ActivationFunctionType.Relu
    Max = mybir.AluOpType.max
    Mult = mybir.AluOpType.mult

    with tile.TileContext(nc) as tc:
        with (
            tc.tile_pool(name="const", bufs=1) as cpool,
            tc.tile_pool(name="att", bufs=16) as apool,
            tc.tile_pool(name="out", bufs=3) as opool,
            tc.tile_pool(name="ps_s", bufs=2, space="PSUM") as ps_s,
            tc.tile_pool(name="ps_pv", bufs=2, space="PSUM") as ps_pv,
        ):
            qa_s = cpool.tile([68, HEADS * NQ], F16)
            ka_s = cpool.tile([68, NTRI * 128], F16)
            vt_s = cpool.tile([128, HEADS * MT * EMBED], F16)
            for i in range(3):
                nc.sync.dma_start(ka_s[32 * i:32 * i + 4, :],
                                  ka[4 * i:4 * i + 4, :])
                nc.sync.dma_start(qa_s[32 * i:32 * i + 4, :],
                                  qa[4 * i:4 * i + 4, :])
            for h in range(HEADS):
                nc.sync.dma_start(
                    vt_s[:, h * MT * EMBED:(h + 1) * MT * EMBED],
                    vt[:, h * MT * EMBED:(h + 1) * MT * EMBED])

            def emit(hgi, state):
                hg = HGROUPS[hgi]
                pv = ps_pv.tile([128, NG], F32, tag="pv")
                pending = []  # deferred PV matmul passes

                def emit_pv(n):
                    for _ in range(min(n, len(pending))):
                        for (jj, lhsT, rhs, start, stop) in pending.pop(0):
                            nc.tensor.matmul(
                                pv[32 * jj:32 * jj + 32, :], lhsT, rhs,
                                start=start, stop=stop,
                                skip_group_check=True,
                                tile_position=(0, 32 * jj))

                for g in range(NTRI):
                    nmt = 3 if g < NTRI - 1 else 2
                    wid = nmt * 512
                    att = {}
                    for h in hg:
                        for qh in (0, 1):
                            P = ps_s.tile([128, 1536], F32, tag="ps")
                            for j in range(nmt):
                                nc.tensor.matmul(
                                    P[:, j * 512:(j + 1) * 512],
                                    ka_s[32 * j:32 * j + 4,
                                         g * 128:(g + 1) * 128],
                                    qa_s[32 * j:32 * j + 4,
                                         h * NQ + qh * 512:
                                         h * NQ + qh * 512 + 512],
                                    start=True, stop=True,
                                    tile_position=(32 * j, 0))
                            if mode == "scores":
                                continue
                            at = apool.tile([128, 1536], F16, tag="att")
                            # full-tile drains alternate engines; ACT gets
                            # 11/21 (it is ~10% faster than DVE at this FD)
                            if (state["drain"] * 11) % 21 < 11:
                                nc.scalar.activation(at[:, 0:wid],
                                                     P[:, 0:wid],
                                                     Relu, scale=ASCALE)
                            else:
                                nc.vector.tensor_scalar(
                                    at[:, 0:wid], P[:, 0:wid],
                                    0.0, ASCALE, Max, Mult)
                            state["drain"] += 1
                            att[(h, qh)] = at
                            emit_pv(2)
                    if mode != "full":
                        continue
                    # enqueue PV for this triple (consumed interleaved)
                    for j in range(nmt):
                        mt = 3 * g + j
                        for bqh in (0, 1):
                            for bdx in (0, 1):
                                grp = []
                                for jj, h in enumerate(hg):
                                    at = att[(h, bqh)]
                                    lhsT = vt_s[:, (h * MT + mt) * EMBED:
                                                (h * MT + mt + 1) * EMBED]
                                    rhs = at[:, j * 512 + bdx * 256:
                                             j * 512 + bdx * 256 + 256]
                                    start = (g == 0 and j == 0
                                             and bqh == 0 and bdx == 0)
                                    stop = (g == NTRI - 1 and j == nmt - 1
                                            and bqh == 1 and bdx == 1)
                                    grp.append((jj, lhsT, rhs, start, stop))
                                pending.append(grp)
                emit_pv(len(pending))
                oh = opool.tile([128, NG], F16, tag="oh")
                nrow = 32 * len(hg)
                nc.vector.tensor_copy(oh[0:nrow, :], pv[0:nrow, :])
                nc.sync.dma_start(
                    outh[hgi * 128:hgi * 128 + nrow, :], oh[0:nrow, :])

            state = {"drain": 0}
            for _rep in range(repeat):
                for hgi in range(len(HGROUPS)):
                    emit(hgi, state)

    nc.compile()
    _CACHE[("nc", repeat)] = nc
    return nc


def _upsample2x(x):
    """[C,32,32] -> [C,64,64] bilinear, half-pixel centers, edge clamp."""
    C, h, w = x.shape
    idx = np.arange(2 * h)
    pos = 0.5 * idx - 0.25
    lo = np.floor(pos).astype(int)
    frac = (pos - lo).astype(np.float32)
    lo_c = np.clip(lo, 0, h - 1)
    hi_c = np.clip(lo + 1, 0, h - 1)
    y = x[:, lo_c, :] * (1 - frac)[None, :, None] + x[:, hi_c, :] * frac[None, :, None]
    z = y[:, :, lo_c] * (1 - frac)[None, None, :] + y[:, :, hi_c] * frac[None, None, :]
    return z.astype(np.float32)


def kernel(second_frame, first_frame_aligned, second_frame_feat,
           first_frame_feat_aligned, Wq, bq, Wk, bk, Wv, bv, Wp, bp, Wt, bt):
    second_frame = np.asarray(second_frame, np.float32)
    first_frame_aligned = np.asarray(first_frame_aligned, np.float32)
    second_frame_feat = np.asarray(second_frame_feat, np.float32)
    first_frame_feat_aligned = np.asarray(first_frame_feat_aligned, np.float32)
    Wq = np.asarray(Wq, np.float32); bq = np.asarray(bq, np.float32)
    Wk = np.asarray(Wk, np.float32); bk = np.asarray(bk, np.float32)
    Wv = np.asarray(Wv, np.float32); bv = np.asarray(bv, np.float32)
    Wp = np.asarray(Wp, np.float32); bp = np.asarray(bp, np.float32)
    Wt = np.asarray(Wt, np.float32); bt = np.asarray(bt, np.float32)

    nc = _build()

    s32 = math.sqrt(EMBED)
    wt = Wt[0]                                   # [32], shared across heads
    Wqh = Wq.reshape(HEADS, EMBED, 3)
    Wkh = Wk.reshape(HEADS, EMBED, 3)
    bqh = bq.reshape(HEADS, EMBED)
    bkh = bk.reshape(HEADS, EMBED)

    # per-head affine pieces of scores = q~ . k~  (k~ = [x1; 1])
    A = np.einsum('hdc,hde->hce', Wqh, Wkh) / s32          # [9,3,3]
    u = np.einsum('hde,hd->he', Wkh, bqh) / s32            # [9,3]
    w = np.einsum('hdc,hd->hc', Wqh, bkh / s32 - wt[None, :])  # [9,3]
    c = (np.einsum('hd,hd->h', bqh, bkh) / s32
         - bqh @ wt - bt[0])                               # [9]

    # per-batch host prep ----------------------------------------------------
    ka_b, qt_b, vt_b = [], [], []
    vsc = (2.0 ** 7) / (TEMPERATURE * 4.0)
    for b in range(B):
        x1 = first_frame_aligned[b].reshape(3, HW)
        x2 = second_frame[b].reshape(3, HW)
        fu = _upsample2x(first_frame_feat_aligned[b]).reshape(4, HW)

        # k~ = [x1; 1], rearranged so row group j holds key chunk 3g+j at
        # column block g (3 concurrent row-tiled matmuls per pass)
        k4 = np.empty((4, HW), np.float32)
        k4[0:3] = x1
        k4[3] = 1.0
        ka = np.zeros((12, NTRI * 128), np.float16)
        for g in range(NTRI):
            for j in range(3 if g < NTRI - 1 else 2):
                ka[4 * j:4 * j + 4, g * 128:(g + 1) * 128] = \
                    k4[:, (3 * g + j) * 128:(3 * g + j + 1) * 128]
        ka_b.append(ka)

        # q~ per head at full 64x64 resolution: [9, 4, H, W]
        q3 = np.einsum('hce,cq->heq', A, x2) + u[:, :, None]   # [9,3,HW]
        q4 = np.einsum('hc,cq->hq', w, x2) + c[:, None]        # [9,HW]
        qt_b.append((q3.reshape(HEADS, 3, H, W),
                     q4.reshape(HEADS, H, W)))

        # V'^T scaled: [p, h, mt, d]
        vtf = (fu.T @ Wv.T + bv[None, :]) * vsc                # [HW, 288]
        vtr = vtf.reshape(MT, 128, HEADS, EMBED)
        vt_b.append(np.ascontiguousarray(
            vtr.transpose(1, 2, 0, 3).reshape(128, HEADS * MT * EMBED)
        ).astype(np.float16))

    in_maps = []
    for cidx in range(N_CORES):
        b, y0 = cidx // 4, (cidx % 4) * 16
        q3, q4 = qt_b[b]
        # grouped query order: (dy, dx, y', x'); replicated on 3 row groups
        qa1 = np.empty((4, HEADS * NQ), np.float16)
        q3c = q3[:, :, y0:y0 + 16, :].reshape(HEADS, 3, 8, 2, 32, 2)
        q3g = q3c.transpose(0, 1, 3, 5, 2, 4).reshape(HEADS, 3, NQ)
        q4c = q4[:, y0:y0 + 16, :].reshape(HEADS, 8, 2, 32, 2)
        q4g = q4c.transpose(0, 2, 4, 1, 3).reshape(HEADS, NQ)
        for h in range(HEADS):
            qa1[0:3, h * NQ:(h + 1) * NQ] = q3g[h]
            qa1[3, h * NQ:(h + 1) * NQ] = q4g[h]
        qa = np.concatenate([qa1, qa1, qa1], axis=0)
        in_maps.append({"qa": qa, "ka": ka_b[b], "vt": vt_b[b]})

    res = bass_utils.run_bass_kernel_spmd(nc, in_maps,
                                          core_ids=list(range(N_CORES)))

    out_ofd = np.zeros((B, 4, 32, 32), np.float32)
    for cidx in range(N_CORES):
        b, y0 = cidx // 4, (cidx % 4) * 16
        oh = res.results[cidx]["outh"].astype(np.float32)   # [288, 256]
        ofd = Wp @ oh + bp[:, None]                         # [4, 256]
        out_ofd[b, :, y0 // 2: y0 // 2 + 8, :] = ofd.reshape(4, 8, 32)
    out_fin = second_frame_feat + out_ofd
    return out_fin, out_ofd


# revision 18
# speedup vs baseline: 1.9235x; 1.3134x over previous
"""Trainium2 Bass kernel for CrossFrameAbsoluteAttn (v2: rank-4 scores + PV pool-fold).

Math (per batch b, head h, d=32, HW=4096):
    Q = Wq x2 + bq ; K = Wk x1 + bk ; V = Wv up(feat1) + bv
    scores = (Q^T K)/sqrt(d) - (Q^T wt + bt)     # [HW_q, HW_k]
    attn = relu(scores)/3000                     # no row normalization
    out = attn V^T ; out_feat = Wp out + bp
    ofd = avgpool2x2(out_feat) ; final = feat2 + ofd

Key structure exploited here:
  * Q and K are affine in the 3-channel frames, so scores factor through a
    4-dim contraction: scores_h[q,m] = q~_h[:,q] . k~[:,m] with
      k~[m]   = [x1_m; 1]                        (shared by ALL 9 heads)
      q~_h[q] = [A_h^T x2_q + u_h ; w_h.x2_q + c_h]
    where A_h = Wq_h^T Wk_h/s, u_h = Wk_h^T bq_h/s, w_h = Wq_h^T(bk_h/s - wt),
    c_h = bq_h.(bk_h/s) - wt.bq_h - bt, s = sqrt(32). K=4 matmuls (cost is
    set by streamed columns only, so the tiny contraction is free).
  * The 2x2 avgpool over queries commutes with everything after the relu, and
    is FOLDED INTO the PV matmul: with queries ordered (dy, dx, y', x'), the
    4 pool members sit at the same position of 4 contiguous 256-col blocks,
    which accumulate into one PSUM tile across the PV matmuls. Zero vector
    cost for pooling.
  * The kernel is PSUM-drain-bound: every score must leave PSUM through
    ScalarE (ACT) or VectorE (DVE), both ~1 elem/lane/cycle for f32 PSUM
    reads. Scores are produced into 3-bank [128,1536] PSUM tiles so the two
    relu drains run at large free-dim (low per-op overhead), split
    ACT [0:768] / DVE [768:1536].
  * PV uses PE column tiling: 4 heads' PV matmuls issued to distinct 32-col
    groups (tile_position=(0,32j)) run concurrently; PV is interleaved
    between score passes to hide in the PE's drain-wait gaps.
  * relu scale 2^-7 keeps attn in fp16 normal range; V carries 2^7/(3000*4).
  * host side: tiny per-head 4-dim affine prep, upsample, and the final 4x288
    projection (as in the reference harness contract).

Sharding: 8 cores split the (batch, query-row) space: core c owns batch c//4,
query rows 16*(c%4)..16*(c%4)+15 (1024 queries), all 9 heads. No collectives.
"""

import math
import sys

import numpy as np

sys.path.insert(0, "/opt/trn_rl_repo")

import concourse.bass as bass  # noqa: E402
import concourse.tile as tile  # noqa: E402
from concourse import bacc, mybir  # noqa: E402
from concourse import bass_utils  # noqa: E402

HEADS = 9
EMBED = 32
TEMPERATURE = 3000.0
B, H, W = 2, 64, 64
HW = H * W
N_CORES = 8
NQ = 1024            # queries per core
NG = NQ // 4         # pooled outputs per core
MT = HW // 128       # 32 key m-tiles
NTRI = 11            # m-tile triples (last has 2)
ASCALE = 2.0 ** -7   # attn fp16 scale (relu side)

F16 = mybir.dt.float16
F32 = mybir.dt.float32

HGROUPS = [(0, 1, 2, 3), (4, 5, 6, 7), (8,)]

_CACHE = {}


def _build(repeat=1, mode="full"):
    # mode: "full" | "drain" (no PV) | "scores" (matmuls only) — for timing
    # component isolation; only "full" computes the real output.
    if ("nc", repeat, mode) in _CACHE:
        return _CACHE[("nc", repeat, mode)]

    nc = bacc.Bacc("TRN2", target_bir_lowering=False, debug=False,
                   num_devices=N_CORES)

    # q~/k~ replicated on 3 partition groups (0-3, 32-35, 64-67) so the 3
    # score matmuls of a pass go to distinct PE row groups and run
    # concurrently (row tiling).
    qa = nc.dram_tensor("qa", [12, HEADS * NQ], F16, kind="ExternalInput").ap()
    ka = nc.dram_tensor("ka", [12, NTRI * 128], F16, kind="ExternalInput").ap()
    vt = nc.dram_tensor("vt", [128, HEADS * MT * EMBED], F16,
                        kind="ExternalInput").ap()
    # rows 0:128 heads 0-3, 128:256 heads 4-7, 256:384 head 8 as 4 partial
    # sums (PV col-split over key residues; host adds the 4 blocks)
    outh = nc.dram_tensor("outh", [384, NG], F16,
                          kind="ExternalOutput").ap()

    Relu = mybir.ActivationFunctionType.Relu
    Max = mybir.AluOpType.max
    Mult = mybir.AluOpType.mult

    with tile.TileContext(nc) as tc:
        with (
            tc.tile_pool(name="const", bufs=1) as cpool,
            tc.tile_pool(name="att", bufs=16) as apool,
            tc.tile_pool(name="out", bufs=3) as opool,
            tc.tile_pool(name="ps_s", bufs=2, space="PSUM") as ps_s,
            tc.tile_pool(name="ps_pv", bufs=2, space="PSUM") as ps_pv,
        ):
            qa_s = cpool.tile([68, HEADS * NQ], F16)
            ka_s = cpool.tile([68, NTRI * 128], F16)
            vt_s = cpool.tile([128, HEADS * MT * EMBED], F16)
            for i in range(3):
                nc.sync.dma_start(ka_s[32 * i:32 * i + 4, :],
                                  ka[4 * i:4 * i + 4, :])
                nc.sync.dma_start(qa_s[32 * i:32 * i + 4, :],
                                  qa[4 * i:4 * i + 4, :])
            for h in range(HEADS):
                nc.sync.dma_start(
                    vt_s[:, h * MT * EMBED:(h + 1) * MT * EMBED],
                    vt[:, h * MT * EMBED:(h + 1) * MT * EMBED])

            def emit(hgi, state):
                hg = HGROUPS[hgi]
                if mode == "full":
                    pv = ps_pv.tile([128, NG], F32, tag="pv")
                else:
                    pv = None
                pending = []  # deferred PV matmul passes

                def emit_pv(n):
                    for _ in range(min(n, len(pending))):
                        for (jj, lhsT, rhs, start, stop) in pending.pop(0):
                            nc.tensor.matmul(
                                pv[32 * jj:32 * jj + 32, :], lhsT, rhs,
                                start=start, stop=stop,
                                skip_group_check=True,
                                tile_position=(0, 32 * jj))

                for g in range(NTRI):
                    nmt = 3 if g < NTRI - 1 else 2
                    wid = nmt * 512
                    att = {}
                    for h in hg:
                        for qh in (0, 1):
                            P = ps_s.tile([128, 1536], F32, tag="ps")
                            for j in range(nmt):
                                nc.tensor.matmul(
                                    P[:, j * 512:(j + 1) * 512],
                                    ka_s[32 * j:32 * j + 4,
                                         g * 128:(g + 1) * 128],
                                    qa_s[32 * j:32 * j + 4,
                                         h * NQ + qh * 512:
                                         h * NQ + qh * 512 + 512],
                                    start=True, stop=True,
                                    tile_position=(32 * j, 0))
                            if mode == "scores":
                                continue
                            at = apool.tile([128, 1536], F16, tag="att")
                            # full-tile drains alternate engines; ACT gets
                            # 12/22 (it is ~20% faster than DVE at this FD)
                            if (state["drain"] * 12) % 22 < 12:
                                nc.scalar.activation(at[:, 0:wid],
                                                     P[:, 0:wid],
                                                     Relu, scale=ASCALE)
                            else:
                                nc.vector.tensor_scalar(
                                    at[:, 0:wid], P[:, 0:wid],
                                    0.0, ASCALE, Max, Mult)
                            state["drain"] += 1
                            att[(h, qh)] = at
                            emit_pv(2)
                    if mode != "full":
                        continue
                    # enqueue PV for this triple (consumed interleaved)
                    for j in range(nmt):
                        mt = 3 * g + j
                        for bqh in (0, 1):
                            for bdx in (0, 1):
                                grp = []
                                if len(hg) == 1:
                                    # single head: col-split PV over key
                                    # residues mt%4 (host adds the 4 blocks)
                                    h = hg[0]
                                    at = att[(h, bqh)]
                                    lhsT = vt_s[:, (h * MT + mt) * EMBED:
                                                (h * MT + mt + 1) * EMBED]
                                    rhs = at[:, j * 512 + bdx * 256:
                                             j * 512 + bdx * 256 + 256]
                                    start = (mt < 4 and bqh == 0 and bdx == 0)
                                    stop = (mt >= MT - 4
                                            and bqh == 1 and bdx == 1)
                                    grp.append((mt % 4, lhsT, rhs,
                                                start, stop))
                                else:
                                    for jj, h in enumerate(hg):
                                        at = att[(h, bqh)]
                                        lhsT = vt_s[:, (h * MT + mt) * EMBED:
                                                    (h * MT + mt + 1) * EMBED]
                                        rhs = at[:, j * 512 + bdx * 256:
                                                 j * 512 + bdx * 256 + 256]
                                        start = (g == 0 and j == 0
                                                 and bqh == 0 and bdx == 0)
                                        stop = (g == NTRI - 1 and j == nmt - 1
                                                and bqh == 1 and bdx == 1)
                                        grp.append((jj, lhsT, rhs,
                                                    start, stop))
                                pending.append(grp)
                emit_pv(len(pending))
                oh = opool.tile([128, NG], F16, tag="oh")
                if mode == "full":
                    nc.vector.tensor_copy(oh[:], pv[:])
                else:
                    nc.vector.memset(oh[:], 0.0)
                nc.sync.dma_start(
                    outh[hgi * 128:(hgi + 1) * 128, :], oh[:])

            state = {"drain": 0}
            for _rep in range(repeat):
                for hgi in range(len(HGROUPS)):
                    emit(hgi, state)

    nc.compile()
    _CACHE[("nc", repeat)] = nc
    return nc


def _upsample2x(x):
    """[C,32,32] -> [C,64,64] bilinear, half-pixel centers, edge clamp."""
    C, h, w = x.shape
    idx = np.arange(2 * h)
    pos = 0.5 * idx - 0.25
    lo = np.floor(pos).astype(int)
    frac = (pos - lo).astype(np.float32)
    lo_c = np.clip(lo, 0, h - 1)
    hi_c = np.clip(lo + 1, 0, h - 1)
    y = x[:, lo_c, :] * (1 - frac)[None, :, None] + x[:, hi_c, :] * frac[None, :, None]
    z = y[:, :, lo_c] * (1 - frac)[None, None, :] + y[:, :, hi_c] * frac[None, None, :]
    return z.astype(np.float32)


def kernel(second_frame, first_frame_aligned, second_frame_feat,
           first_frame_feat_aligned, Wq, bq, Wk, bk, Wv, bv, Wp, bp, Wt, bt):
    second_frame = np.asarray(second_frame, np.float32)
    first_frame_aligned = np.asarray(first_frame_aligned, np.float32)
    second_frame_feat = np.asarray(second_frame_feat, np.float32)
    first_frame_feat_aligned = np.asarray(first_frame_feat_aligned, np.float32)
    Wq = np.asarray(Wq, np.float32); bq = np.asarray(bq, np.float32)
    Wk = np.asarray(Wk, np.float32); bk = np.asarray(bk, np.float32)
    Wv = np.asarray(Wv, np.float32); bv = np.asarray(bv, np.float32)
    Wp = np.asarray(Wp, np.float32); bp = np.asarray(bp, np.float32)
    Wt = np.asarray(Wt, np.float32); bt = np.asarray(bt, np.float32)

    nc = _build()

    s32 = math.sqrt(EMBED)
    wt = Wt[0]                                   # [32], shared across heads
    Wqh = Wq.reshape(HEADS, EMBED, 3)
    Wkh = Wk.reshape(HEADS, EMBED, 3)
    bqh = bq.reshape(HEADS, EMBED)
    bkh = bk.reshape(HEADS, EMBED)

    # per-head affine pieces of scores = q~ . k~  (k~ = [x1; 1])
    A = np.einsum('hdc,hde->hce', Wqh, Wkh) / s32          # [9,3,3]
    u = np.einsum('hde,hd->he', Wkh, bqh) / s32            # [9,3]
    w = np.einsum('hdc,hd->hc', Wqh, bkh / s32 - wt[None, :])  # [9,3]
    c = (np.einsum('hd,hd->h', bqh, bkh) / s32
         - bqh @ wt - bt[0])                               # [9]

    # per-batch host prep ----------------------------------------------------
    ka_b, qt_b, vt_b = [], [], []
    vsc = (2.0 ** 7) / (TEMPERATURE * 4.0)
    for b in range(B):
        x1 = first_frame_aligned[b].reshape(3, HW)
        x2 = second_frame[b].reshape(3, HW)
        fu = _upsample2x(first_frame_feat_aligned[b]).reshape(4, HW)

        # k~ = [x1; 1], rearranged so row group j holds key chunk 3g+j at
        # column block g (3 concurrent row-tiled matmuls per pass)
        k4 = np.empty((4, HW), np.float32)
        k4[0:3] = x1
        k4[3] = 1.0
        ka = np.zeros((12, NTRI * 128), np.float16)
        for g in range(NTRI):
            for j in range(3 if g < NTRI - 1 else 2):
                ka[4 * j:4 * j + 4, g * 128:(g + 1) * 128] = \
                    k4[:, (3 * g + j) * 128:(3 * g + j + 1) * 128]
        ka_b.append(ka)

        # q~ per head at full 64x64 resolution: [9, 4, H, W]
        q3 = np.einsum('hce,cq->heq', A, x2) + u[:, :, None]   # [9,3,HW]
        q4 = np.einsum('hc,cq->hq', w, x2) + c[:, None]        # [9,HW]
        qt_b.append((q3.reshape(HEADS, 3, H, W),
                     q4.reshape(HEADS, H, W)))

        # V'^T scaled: [p, h, mt, d]
        vtf = (fu.T @ Wv.T + bv[None, :]) * vsc                # [HW, 288]
        vtr = vtf.reshape(MT, 128, HEADS, EMBED)
        vt_b.append(np.ascontiguousarray(
            vtr.transpose(1, 2, 0, 3).reshape(128, HEADS * MT * EMBED)
        ).astype(np.float16))

    in_maps = []
    for cidx in range(N_CORES):
        b, y0 = cidx // 4, (cidx % 4) * 16
        q3, q4 = qt_b[b]
        # grouped query order: (dy, dx, y', x'); replicated on 3 row groups
        qa1 = np.empty((4, HEADS * NQ), np.float16)
        q3c = q3[:, :, y0:y0 + 16, :].reshape(HEADS, 3, 8, 2, 32, 2)
        q3g = q3c.transpose(0, 1, 3, 5, 2, 4).reshape(HEADS, 3, NQ)
        q4c = q4[:, y0:y0 + 16, :].reshape(HEADS, 8, 2, 32, 2)
        q4g = q4c.transpose(0, 2, 4, 1, 3).reshape(HEADS, NQ)
        for h in range(HEADS):
            qa1[0:3, h * NQ:(h + 1) * NQ] = q3g[h]
            qa1[3, h * NQ:(h + 1) * NQ] = q4g[h]
        qa = np.concatenate([qa1, qa1, qa1], axis=0)
        in_maps.append({"qa": qa, "ka": ka_b[b], "vt": vt_b[b]})

    res = bass_utils.run_bass_kernel_spmd(nc, in_maps,
                                          core_ids=list(range(N_CORES)))

    out_ofd = np.zeros((B, 4, 32, 32), np.float32)
    for cidx in range(N_CORES):
        b, y0 = cidx // 4, (cidx % 4) * 16
        ohr = res.results[cidx]["outh"].astype(np.float32)  # [384, 256]
        oh = np.empty((288, NG), np.float32)
        oh[0:256] = ohr[0:256]
        oh[256:288] = (ohr[256:288] + ohr[288:320]
                       + ohr[320:352] + ohr[352:384])
        ofd = Wp @ oh + bp[:, None]                         # [4, 256]
        out_ofd[b, :, y0 // 2: y0 // 2 + 8, :] = ofd.reshape(4, 8, 32)
    out_fin = second_frame_feat + out_ofd
    return out_fin, out_ofd
